# revision 10
# baseline (speedup 1.0000x reference)
"""BoundaryLoss kernel for 8 Trainium2 NeuronCores (v2).

loss = sum_c mean_{b,h,w}((|sobel(labels_c)| - |sobel(probs_c)|)^2)
     = (Sigma_m - 2*Sigma_sqrt(q)) / (B*H*W),   m = gx^2+gy^2, q = m_l*m_p

Data-parallel: core k processes batches [2k, 2k+1] x classes 1..4
(8 image pairs of 512x512). Partial sums are combined on host.

v2 changes vs v1:
- Inputs cast to fp8e4 on host (exact-weight Sobel taps; ~0.5% loss bias,
  well inside the 2e-2 gate). Conv matmuls use fp8 DoubleRowSwInterleave
  perf mode: each matmul contracts TWO (weights, moving) pairs at 2 rows/
  cycle, so the 4 gradient fields cost 6 half-rate matmuls instead of 9
  full-rate ones (PE time ~3x lower; HBM traffic halved).
  Moving pairs must use i-stride 2 (stride-1 pairs hard-fault the PE):
    gx   = DR(bv | -bv  ; x[c-1], x[c+1])
    gy   = DR(sdf | sdf ; x[c-1], x[c+1]) + DR(2sdf | 0 ; x[c], x[c+2])
  Weights are host-interleaved: phys[:, 2j] = W0[:, 127-j],
  phys[:, 2j+1] = W1[:, 127-j] (the documented SwInterleave layout).
- PSUM: gB = [gx_l | gx_p | gy_l] (3 banks, 2 bufs) + gS = [gy_p]
  (1 bank, single-buffered) + accP (1 bank) = 8 banks.
- Evacuation split tuned from the engine cost model: ACT Squares
  gB[0:EA] in one op; DVE copies the rest to fp16; Pool squares the
  copies (GPSIMD cannot read PSUM). m-add split DVE/Pool; q-mul on DVE.
- Sigma_m via PE ones-matmuls accumulating into accP across all
  iterations (frees ACT/DVE accum reads); Sigma_sqrt(q) via the batched
  ACT Sqrt's accum_out.
"""

import sys

import numpy as np

if "/opt/trn_rl_repo" not in sys.path:
    sys.path.insert(0, "/opt/trn_rl_repo")

from contextlib import ExitStack

import ml_dtypes

import concourse.bass as bass
import concourse.mybir as mybir
import concourse.tile as tile

H = W = 512
N_IMG = 8          # image pairs per core
BAND = 126         # output rows per full band
N_BANDS = 4        # full bands; bottom 8 rows in 1 packed tail iter
N_ITERS = N_IMG * N_BANDS + 1
XW = 1032          # [0 | L:1..512 | 0,0 | P:515..1026 | 0 pads]
LB = 1             # labels block base col
PB = 515           # probs block base col
EA = 1440          # gB cols evacuated by the ACT Square op
NSUF = 1536 - EA   # gB cols copied by DVE (gy_l tail)
NC16 = NSUF + 512  # fp16 copy width (gB suffix + gS)
ADD_DVE = 896      # m-add cols on DVE (rest on Pool)
# sqrt batches: big groups amortize overhead, small ones cut the drain
SQRT_GSIZE = [8, 8, 8, 6, 2, 1]
assert sum(SQRT_GSIZE) == N_ITERS
SQRT_GRP = [g for g, n in enumerate(SQRT_GSIZE) for _ in range(n)]
SQRT_POS = [i for n in SQRT_GSIZE for i in range(n)]
N_SQRT_G = len(SQRT_GSIZE)

F32 = mybir.dt.float32
F16 = mybir.dt.float16
F8 = mybir.dt.float8e4
DRS = mybir.MatmulPerfMode.DoubleRowSwInterleave


def _band_matrices():
    bv = np.zeros((128, 128), np.float32)   # vertical smooth [1,2,1]
    sdf = np.zeros((128, 128), np.float32)  # vertical diff [1,0,-1]
    for c in range(126):
        bv[c, c] = 1.0
        bv[c + 1, c] = 2.0
        bv[c + 2, c] = 1.0
        sdf[c, c] = 1.0
        sdf[c + 2, c] = -1.0
    # Packed tail: image k's rows 503..511 at partitions 10k..10k+8
    # (10k+9 zero halo), outputs 504..511 at partitions 8k..8k+7.
    bvm = np.zeros((128, 128), np.float32)
    sdfm = np.zeros((128, 128), np.float32)
    for k in range(8):
        for i in range(8):
            bvm[10 * k + i, 8 * k + i] = 1.0
            bvm[10 * k + i + 1, 8 * k + i] = 2.0
            bvm[10 * k + i + 2, 8 * k + i] = 1.0
            sdfm[10 * k + i, 8 * k + i] = 1.0
            sdfm[10 * k + i + 2, 8 * k + i] = -1.0
    return bv, sdf, bvm, sdfm


def _ileave(w0, w1):
    """SwInterleave weight layout: pairs interleaved, columns reversed."""
    ph = np.zeros((128, 256), np.float32)
    ph[:, 0::2] = w0[:, ::-1]
    ph[:, 1::2] = w1[:, ::-1]
    return ph


def _stationaries():
    bv, sdf, bvm, sdfm = _band_matrices()
    zz = np.zeros((128, 128), np.float32)
    blocks = [
        _ileave(bv, -bv),          # PX
        _ileave(sdf, sdf),         # PA
        _ileave(2.0 * sdf, zz),    # PB
        _ileave(bvm, -bvm),        # PXm
        _ileave(sdfm, sdfm),       # PAm
        _ileave(2.0 * sdfm, zz),   # PBm
    ]
    return np.concatenate(blocks, axis=1).astype(ml_dtypes.float8_e4m3)


def _split_waits_json(bir: bytes, maxw: int = 1) -> bytes:
    """Walrus in this container rejects instructions with >1 semaphore wait.
    Split extra waits onto NoOp carriers inserted just before the
    instruction on the same engine."""
    import orjson

    d = orjson.loads(bir)
    ctr = 0
    for fn in d["functions"]:
        for b in fn["blocks"]:
            new = []
            for ins in b["instructions"]:
                si = ins.get("sync_info")
                if si:
                    waits = si.get("on_wait") or []
                    if len(waits) > maxw:
                        keep = waits[-maxw:] if maxw else []
                        for w in waits[: len(waits) - maxw]:
                            ctr += 1
                            new.append({
                                "debug": ins.get("debug", 0),
                                "engine": ins["engine"],
                                "ins": [],
                                "outs": [],
                                "name": f"{ins['name']}-wsplit{ctr}",
                                "opcode": "NoOp",
                                "sync_info": {"on_wait": [w], "on_update": []},
                            })
                        si["on_wait"] = keep
                new.append(ins)
            b["instructions"] = new
    return orjson.dumps(d)


def _patch_serialization(nc):
    fixed = _split_waits_json(nc.to_json_bytes())
    nc.to_json_bytes = lambda: fixed
    return nc


def _pair(ap2d, istride=2):
    """[p, N] slice -> [p, 2, N] moving-pair AP (i-stride 2 only: stride-1
    pairs fault the PE in DoubleRow mode)."""
    c = ap2d.unsqueeze(1).copy()
    c.ap[1] = (istride, 2)
    return c


def build_kernel(loop: int = 1):
    nc = bass.Bass()
    x = nc.dram_tensor("x", [N_IMG, H + 1, XW], F8, kind="ExternalInput")
    xtail = nc.dram_tensor("xtail", [80, XW], F8, kind="ExternalInput")
    consts = nc.dram_tensor("consts", [128, 1536], F8, kind="ExternalInput")
    out = nc.dram_tensor("out", [128, 512 + N_SQRT_G], F32,
                         kind="ExternalOutput")

    with ExitStack() as ctx:
        tc = ctx.enter_context(tile.TileContext(nc))
        cpool = ctx.enter_context(tc.tile_pool(name="consts", bufs=1))
        xpool = ctx.enter_context(tc.tile_pool(name="x", bufs=1))
        gb_pool = ctx.enter_context(
            tc.tile_pool(name="gb", bufs=2, space="PSUM"))
        gs_pool = ctx.enter_context(
            tc.tile_pool(name="gs", bufs=1, space="PSUM"))
        ac_pool = ctx.enter_context(
            tc.tile_pool(name="ac", bufs=1, space="PSUM"))
        sq_pool = ctx.enter_context(tc.tile_pool(name="sq", bufs=4))
        c16_pool = ctx.enter_context(tc.tile_pool(name="c16", bufs=3))
        m_pool = ctx.enter_context(tc.tile_pool(name="m", bufs=3))
        qp_pool = ctx.enter_context(tc.tile_pool(name="qp", bufs=2))
        s_pool = ctx.enter_context(tc.tile_pool(name="s", bufs=2))
        acc_pool = ctx.enter_context(tc.tile_pool(name="acc", bufs=1))

        # PE p-state warmup while the first DMAs land.
        warm = cpool.tile([128, 512], F16, tag="warm")
        nc.gpsimd.memset(warm[:, :], 0.0)

        wmat = cpool.tile([128, 1536], F8, tag="wmat")
        nc.scalar.dma_start(out=wmat[:, :], in_=consts[:, :])
        (PX, PA, PBw, PXm, PAm, PBm) = (
            wmat[:, 256 * i:256 * i + 256] for i in range(6))

        ones = cpool.tile([128, 1], F16, tag="ones")
        nc.vector.memset(ones[:, :], 1.0)
        acc_s = acc_pool.tile([128, N_SQRT_G], F32, tag="acc_s")
        nc.vector.memset(acc_s[:, :], 0.0)

        N_XT = 6
        xt = [xpool.tile([128, XW], F8, name=f"x{j}", tag=f"x{j}")
              for j in range(N_XT)]
        xm = xpool.tile([128, XW], F8, tag="xm")

        gwarm = gs_pool.tile([128, 512], F32, tag="gs")
        for _ in range(7):
            nc.tensor.matmul(gwarm[0:126, 0:512], warm[0:128, 0:126],
                             warm[0:128, 0:512], start=True, stop=True)

        accP = ac_pool.tile([128, 512], F32, tag="accP")

        loop_ctx = tc.For_i(0, loop, 1) if loop > 1 else None
        if loop_ctx is not None:
            loop_ctx.__enter__()

        def band_tile(it):
            return xt[it % N_XT] if it < N_IMG * N_BANDS else xm

        def emit_dma(it):
            if it >= N_ITERS:
                return
            if it < N_IMG * N_BANDS:
                img, t = divmod(it, N_BANDS)
                nc.sync.dma_start(
                    out=xt[it % N_XT][0:128, :],
                    in_=x[img, BAND * t:BAND * t + 128, :])
            else:
                nc.sync.dma_start(out=xm[0:80, :], in_=xtail[:, :])

        def emit_mms(gB, gS, xb, it):
            if it < N_IMG * N_BANDS:
                px, pa, pb, kp = PX, PA, PBw, 128
            else:
                px, pa, pb, kp = PXm, PAm, PBm, 80
            for half, c0 in ((0, LB), (1, PB)):
                nc.tensor.matmul(gB[0:128, 512 * half:512 * half + 512],
                                 px[0:kp, :],
                                 _pair(xb[0:kp, c0 - 1:c0 - 1 + 512]),
                                 start=True, stop=True, perf_mode=DRS)
            nc.tensor.matmul(gB[0:128, 1024:1536], pa[0:kp, :],
                             _pair(xb[0:kp, LB - 1:LB - 1 + 512]),
                             start=True, stop=False, perf_mode=DRS)
            nc.tensor.matmul(gB[0:128, 1024:1536], pb[0:kp, :],
                             _pair(xb[0:kp, LB:LB + 512]),
                             start=False, stop=True, perf_mode=DRS)
            nc.tensor.matmul(gS[0:128, 0:512], pa[0:kp, :],
                             _pair(xb[0:kp, PB - 1:PB - 1 + 512]),
                             start=True, stop=False, perf_mode=DRS)
            nc.tensor.matmul(gS[0:128, 0:512], pb[0:kp, :],
                             _pair(xb[0:kp, PB:PB + 512]),
                             start=False, stop=True, perf_mode=DRS)

        # deferred work queues (software pipelining; FIFO, aged entries):
        # pool-squares 1 iter behind, adds+mul 2 behind, Sigma-m 3 behind.
        pend_psq = []     # (sq, c16, it)
        pend_add = []     # (sq, it)
        pend_sum = []     # (m, pv)
        qp_cur = None
        s_cur = None
        qp_done = []      # (qp, pv, ncols, grp)
        first_sum = [True]

        def pv_of(it):
            return BAND if it < N_IMG * N_BANDS else 64

        def emit_psq(ent):
            sq, c16, it = ent
            pv = pv_of(it)
            nc.gpsimd.tensor_tensor(
                out=sq[0:pv, EA:2048], in0=c16[0:pv, 0:NC16],
                in1=c16[0:pv, 0:NC16], op=mybir.AluOpType.mult)
            pend_add.append((sq, it))

        def emit_add(ent):
            nonlocal qp_cur
            sq, it = ent
            pv = pv_of(it)
            m = m_pool.tile([128, 1024], F16, tag="m")
            nc.vector.tensor_add(m[0:pv, 0:ADD_DVE], sq[0:pv, 0:ADD_DVE],
                                 sq[0:pv, 1024:1024 + ADD_DVE])
            nc.gpsimd.tensor_tensor(
                out=m[0:pv, ADD_DVE:1024], in0=sq[0:pv, ADD_DVE:1024],
                in1=sq[0:pv, 1024 + ADD_DVE:2048], op=mybir.AluOpType.add)
            g, pos = SQRT_GRP[it], SQRT_POS[it]
            if pos == 0:
                qp_cur = qp_pool.tile([128, 4096], F16, tag="qp")
            nc.vector.tensor_mul(qp_cur[0:pv, 512 * pos:512 * pos + 512],
                                 m[0:pv, 0:512], m[0:pv, 512:1024])
            if pos == SQRT_GSIZE[g] - 1:
                qp_done.append((qp_cur, pv, 512 * SQRT_GSIZE[g], g))
            pend_sum.append((m, pv))

        def emit_msum(ent):
            m, pv = ent
            for hh in (0, 1):
                st = first_sum[0]
                first_sum[0] = False
                nc.tensor.matmul(accP[0:1, 0:512], ones[0:pv, 0:1],
                                 m[0:pv, 512 * hh:512 * hh + 512],
                                 start=st, stop=False, skip_group_check=True)

        def emit_sqrt(ent):
            nonlocal s_cur
            qp, pv, ncols, grp = ent
            s_cur = s_pool.tile([128, 4096], F16, tag="s")
            nc.scalar.activation(s_cur[0:pv, 0:ncols], qp[0:pv, 0:ncols],
                                 mybir.ActivationFunctionType.Sqrt,
                                 accum_out=acc_s[0:pv, grp:grp + 1])

        emit_dma(0)
        emit_dma(1)
        emit_dma(2)
        for it in range(N_ITERS):
            pv = pv_of(it)
            xb = band_tile(it)
            gB = gb_pool.tile([128, 1536], F32, tag="gB")
            gS = gs_pool.tile([128, 512], F32, tag="gs")
            emit_mms(gB, gS, xb, it)
            if len(pend_sum) >= 2:
                emit_msum(pend_sum.pop(0))
            emit_dma(it + 3)

            # ACT evacuates gB[0:EA] as squares
            sq = sq_pool.tile([128, 2048], F16, tag="sq")
            nc.scalar.activation(sq[0:pv, 0:EA], gB[0:pv, 0:EA],
                                 mybir.ActivationFunctionType.Square)
            # DVE: PSUM evacuation copies first (gS frees its bank)
            c16 = c16_pool.tile([128, NC16], F16, tag="c16")
            nc.vector.tensor_copy(c16[0:pv, NSUF:NC16], gS[0:pv, 0:512])
            nc.vector.tensor_copy(c16[0:pv, 0:NSUF], gB[0:pv, EA:1536])
            if len(pend_add) >= 2:
                emit_add(pend_add.pop(0))
            if pend_psq:
                emit_psq(pend_psq.pop(0))
            pend_psq.append((sq, c16, it))
            if qp_done:
                emit_sqrt(qp_done.pop(0))

        while pend_psq:
            emit_psq(pend_psq.pop(0))
        while pend_add:
            emit_add(pend_add.pop(0))
        while pend_sum:
            emit_msum(pend_sum.pop(0))
        while qp_done:
            emit_sqrt(qp_done.pop(0))
        # close the accP accumulation group
        nc.tensor.matmul(accP[0:1, 0:512], ones[0:1, 0:1], warm[0:1, 0:512],
                         start=False, stop=True, skip_group_check=True)

        if loop_ctx is not None:
            loop_ctx.__exit__(None, None, None)

        accm = acc_pool.tile([128, 512], F32, tag="accm")
        nc.scalar.activation(accm[0:1, 0:512], accP[0:1, 0:512],
                             mybir.ActivationFunctionType.Copy)
        nc.sync.dma_start(out=out[:, 0:N_SQRT_G], in_=acc_s[:, :])
        nc.sync.dma_start(out=out[0:1, N_SQRT_G:N_SQRT_G + 512],
                          in_=accm[0:1, 0:512])
    return _patch_serialization(nc)


def _prep_core_inputs(l_imgs, p_imgs, wmat):
    """l_imgs/p_imgs: [N_IMG, H, W] fp8 arrays for one core."""
    x = np.zeros((N_IMG, H + 1, XW), ml_dtypes.float8_e4m3)
    x[:, 1:, LB:LB + W] = l_imgs
    x[:, 1:, PB:PB + W] = p_imgs
    xtail = np.zeros((80, XW), ml_dtypes.float8_e4m3)
    for k in range(N_IMG):
        xtail[10 * k:10 * k + 9, LB:LB + W] = l_imgs[k, 503:512]
        xtail[10 * k:10 * k + 9, PB:PB + W] = p_imgs[k, 503:512]
    return {"x": x, "xtail": xtail, "consts": wmat}


_NC = None


def kernel(probs, labels):
    global _NC
    from concourse.bass_utils import run_bass_kernel_spmd

    if _NC is None:
        _NC = build_kernel()

    p = np.ascontiguousarray(np.asarray(probs)[:, 1:5]).astype(
        ml_dtypes.float8_e4m3)
    l = np.ascontiguousarray(np.asarray(labels)[:, 1:5]).astype(
        ml_dtypes.float8_e4m3)
    wmat = _stationaries()

    in_maps = []
    for k in range(8):
        in_maps.append(_prep_core_inputs(
            l[2 * k:2 * k + 2].reshape(N_IMG, H, W),
            p[2 * k:2 * k + 2].reshape(N_IMG, H, W), wmat))
    res = run_bass_kernel_spmd(_NC, in_maps, list(range(8)))
    total = 0.0
    for r in res.results:
        o = r["out"].astype(np.float64)
        total += (o[0, N_SQRT_G:].sum() - 2.0 * o[:, 0:N_SQRT_G].sum())
    return np.float32(total / (16 * H * W))


# revision 33
# speedup vs baseline: 1.1126x; 1.1126x over previous
"""BoundaryLoss kernel for 8 Trainium2 NeuronCores (v2).

loss = sum_c mean_{b,h,w}((|sobel(labels_c)| - |sobel(probs_c)|)^2)
     = (Sigma_m - 2*Sigma_sqrt(q)) / (B*H*W),   m = gx^2+gy^2, q = m_l*m_p

Data-parallel: core k processes batches [2k, 2k+1] x classes 1..4
(8 image pairs of 512x512). Partial sums are combined on host.

v2 changes vs v1:
- Inputs cast to fp8e4 on host (exact-weight Sobel taps; ~0.5% loss bias,
  well inside the 2e-2 gate). Conv matmuls use fp8 DoubleRowSwInterleave
  perf mode: each matmul contracts TWO (weights, moving) pairs at 2 rows/
  cycle, so the 4 gradient fields cost 6 half-rate matmuls instead of 9
  full-rate ones (PE time ~3x lower; HBM traffic halved).
  Moving pairs must use i-stride 2 (stride-1 pairs hard-fault the PE):
    gx   = DR(bv | -bv  ; x[c-1], x[c+1])
    gy   = DR(sdf | sdf ; x[c-1], x[c+1]) + DR(2sdf | 0 ; x[c], x[c+2])
  Weights are host-interleaved: phys[:, 2j] = W0[:, 127-j],
  phys[:, 2j+1] = W1[:, 127-j] (the documented SwInterleave layout).
- PSUM: gB = [gx_l | gx_p | gy_l] (3 banks, 2 bufs) + gS = [gy_p]
  (1 bank, single-buffered) + accP (1 bank) = 8 banks.
- Evacuation split tuned from the engine cost model: ACT Squares
  gB[0:EA] in one op; DVE copies the rest to fp16; Pool squares the
  copies (GPSIMD cannot read PSUM). m-add split DVE/Pool; q-mul on DVE.
- Sigma_m via PE ones-matmuls accumulating into accP across all
  iterations (frees ACT/DVE accum reads); Sigma_sqrt(q) via the batched
  ACT Sqrt's accum_out.
"""

import sys

import numpy as np

if "/opt/trn_rl_repo" not in sys.path:
    sys.path.insert(0, "/opt/trn_rl_repo")

from contextlib import ExitStack

import ml_dtypes

import concourse.bass as bass
import concourse.mybir as mybir
import concourse.tile as tile

H = W = 512
N_IMG = 8          # image pairs per core
BAND = 126         # output rows per full band
N_BANDS = 4        # full bands; bottom 8 rows in 1 packed tail iter
N_ITERS = N_IMG * N_BANDS + 1
XW = 1032          # [0 | L:1..512 | 0,0 | P:515..1026 | 0 pads]
LB = 1             # labels block base col
PB = 515           # probs block base col
EA = 1536          # gB cols evacuated by the ACT Square op
NSUF = 1536 - EA   # gB cols copied by DVE (gy_l tail)
NC16 = NSUF + 512  # fp16 copy width (gB suffix + gS)
ADD_DVE = 928      # m-add cols on DVE (rest on Pool)
# sqrt batches: big groups amortize overhead, small ones cut the drain
SQRT_GSIZE = [8, 8, 8, 6, 2, 1]
assert sum(SQRT_GSIZE) == N_ITERS
SQRT_GRP = [g for g, n in enumerate(SQRT_GSIZE) for _ in range(n)]
SQRT_POS = [i for n in SQRT_GSIZE for i in range(n)]
N_SQRT_G = len(SQRT_GSIZE)
N_ACCS = 2 * N_SQRT_G

F32 = mybir.dt.float32
F16 = mybir.dt.float16
F8 = mybir.dt.float8e4
DRS = mybir.MatmulPerfMode.DoubleRowSwInterleave


def _band_matrices():
    bv = np.zeros((128, 128), np.float32)   # vertical smooth [1,2,1]
    sdf = np.zeros((128, 128), np.float32)  # vertical diff [1,0,-1]
    for c in range(126):
        bv[c, c] = 1.0
        bv[c + 1, c] = 2.0
        bv[c + 2, c] = 1.0
        sdf[c, c] = 1.0
        sdf[c + 2, c] = -1.0
    # Packed tail: image k's rows 503..511 at partitions 10k..10k+8
    # (10k+9 zero halo), outputs 504..511 at partitions 8k..8k+7.
    bvm = np.zeros((128, 128), np.float32)
    sdfm = np.zeros((128, 128), np.float32)
    for k in range(8):
        for i in range(8):
            bvm[10 * k + i, 8 * k + i] = 1.0
            bvm[10 * k + i + 1, 8 * k + i] = 2.0
            bvm[10 * k + i + 2, 8 * k + i] = 1.0
            sdfm[10 * k + i, 8 * k + i] = 1.0
            sdfm[10 * k + i + 2, 8 * k + i] = -1.0
    return bv, sdf, bvm, sdfm


def _ileave(w0, w1):
    """SwInterleave weight layout: pairs interleaved, columns reversed."""
    ph = np.zeros((128, 256), np.float32)
    ph[:, 0::2] = w0[:, ::-1]
    ph[:, 1::2] = w1[:, ::-1]
    return ph


def _stationaries():
    bv, sdf, bvm, sdfm = _band_matrices()
    zz = np.zeros((128, 128), np.float32)
    blocks = [
        _ileave(bv, -bv),          # PX
        _ileave(sdf, sdf),         # PA
        _ileave(2.0 * sdf, zz),    # PB
        _ileave(bvm, -bvm),        # PXm
        _ileave(sdfm, sdfm),       # PAm
        _ileave(2.0 * sdfm, zz),   # PBm
    ]
    return np.concatenate(blocks, axis=1).astype(ml_dtypes.float8_e4m3)


def _split_waits_json(bir: bytes, maxw: int = 1) -> bytes:
    """Walrus in this container rejects instructions with >1 semaphore wait.
    Split extra waits onto NoOp carriers inserted just before the
    instruction on the same engine."""
    import orjson

    d = orjson.loads(bir)
    ctr = 0
    for fn in d["functions"]:
        for b in fn["blocks"]:
            new = []
            for ins in b["instructions"]:
                si = ins.get("sync_info")
                if si:
                    waits = si.get("on_wait") or []
                    if len(waits) > maxw:
                        keep = waits[-maxw:] if maxw else []
                        for w in waits[: len(waits) - maxw]:
                            ctr += 1
                            new.append({
                                "debug": ins.get("debug", 0),
                                "engine": ins["engine"],
                                "ins": [],
                                "outs": [],
                                "name": f"{ins['name']}-wsplit{ctr}",
                                "opcode": "NoOp",
                                "sync_info": {"on_wait": [w], "on_update": []},
                            })
                        si["on_wait"] = keep
                new.append(ins)
            b["instructions"] = new
    return orjson.dumps(d)


def _patch_serialization(nc):
    fixed = _split_waits_json(nc.to_json_bytes())
    nc.to_json_bytes = lambda: fixed
    return nc


def _pair(ap2d, istride=2):
    """[p, N] slice -> [p, 2, N] moving-pair AP (i-stride 2 only: stride-1
    pairs fault the PE in DoubleRow mode)."""
    c = ap2d.unsqueeze(1).copy()
    c.ap[1] = (istride, 2)
    return c


def build_kernel(loop: int = 1):
    nc = bass.Bass()
    x = nc.dram_tensor("x", [N_IMG, H + 1, XW], F8, kind="ExternalInput")
    xtail = nc.dram_tensor("xtail", [80, XW], F8, kind="ExternalInput")
    consts = nc.dram_tensor("consts", [128, 1536], F8, kind="ExternalInput")
    out = nc.dram_tensor("out", [128, 512 + N_ACCS + N_ITERS], F32,
                         kind="ExternalOutput")

    with ExitStack() as ctx:
        tc = ctx.enter_context(tile.TileContext(nc))
        cpool = ctx.enter_context(tc.tile_pool(name="consts", bufs=1))
        xpool = ctx.enter_context(tc.tile_pool(name="x", bufs=1))
        gb_pool = ctx.enter_context(
            tc.tile_pool(name="gb", bufs=2, space="PSUM"))
        gs_pool = ctx.enter_context(
            tc.tile_pool(name="gs", bufs=1, space="PSUM"))
        ac_pool = ctx.enter_context(
            tc.tile_pool(name="ac", bufs=1, space="PSUM"))
        sq_pool = ctx.enter_context(tc.tile_pool(name="sq", bufs=5))
        c16_pool = ctx.enter_context(tc.tile_pool(name="c16", bufs=4))
        m_pool = ctx.enter_context(tc.tile_pool(name="m", bufs=4))
        qp_pool = ctx.enter_context(tc.tile_pool(name="qp", bufs=3))
        s_pool = ctx.enter_context(tc.tile_pool(name="s", bufs=3))
        acc_pool = ctx.enter_context(tc.tile_pool(name="acc", bufs=1))

        # PE p-state warmup while the first DMAs land.
        warm = cpool.tile([128, 512], F16, tag="warm")
        nc.gpsimd.memset(warm[:, :], 0.0)

        wmat = cpool.tile([128, 1536], F8, tag="wmat")
        nc.scalar.dma_start(out=wmat[:, :], in_=consts[:, :])
        (PX, PA, PBw, PXm, PAm, PBm) = (
            wmat[:, 256 * i:256 * i + 256] for i in range(6))

        ones = cpool.tile([128, 1], F16, tag="ones")
        nc.vector.memset(ones[:, :], 1.0)
        acc_s = acc_pool.tile([128, N_ACCS], F32, tag="acc_s")
        nc.vector.memset(acc_s[:, :], 0.0)
        acc_a = acc_pool.tile([128, N_ITERS], F32, tag="acc_a")
        nc.vector.memset(acc_a[:, :], 0.0)

        N_XT = 6
        xt = [xpool.tile([128, XW], F8, name=f"x{j}", tag=f"x{j}")
              for j in range(N_XT)]
        xm = xpool.tile([128, XW], F8, tag="xm")

        gwarm = gs_pool.tile([128, 512], F32, tag="gs")
        for _ in range(7):
            nc.tensor.matmul(gwarm[0:126, 0:512], warm[0:128, 0:126],
                             warm[0:128, 0:512], start=True, stop=True)

        accP = ac_pool.tile([128, 512], F32, tag="accP")

        loop_ctx = tc.For_i(0, loop, 1) if loop > 1 else None
        if loop_ctx is not None:
            loop_ctx.__enter__()

        def band_tile(it):
            return xt[it % N_XT] if it < N_IMG * N_BANDS else xm

        def emit_dma(it):
            if it >= N_ITERS:
                return
            if it < N_IMG * N_BANDS:
                img, t = divmod(it, N_BANDS)
                nc.sync.dma_start(
                    out=xt[it % N_XT][0:128, :],
                    in_=x[img, BAND * t:BAND * t + 128, :])
            else:
                nc.sync.dma_start(out=xm[0:80, :], in_=xtail[:, :])

        def emit_mms(gB, gS, xb, it):
            if it < N_IMG * N_BANDS:
                px, pa, pb, kp = PX, PA, PBw, 128
            else:
                px, pa, pb, kp = PXm, PAm, PBm, 80
            for half, c0 in ((0, LB), (1, PB)):
                nc.tensor.matmul(gB[0:128, 512 * half:512 * half + 512],
                                 px[0:kp, :],
                                 _pair(xb[0:kp, c0 - 1:c0 - 1 + 512]),
                                 start=True, stop=True, perf_mode=DRS)
            nc.tensor.matmul(gB[0:128, 1024:1536], pa[0:kp, :],
                             _pair(xb[0:kp, LB - 1:LB - 1 + 512]),
                             start=True, stop=False, perf_mode=DRS)
            nc.tensor.matmul(gB[0:128, 1024:1536], pb[0:kp, :],
                             _pair(xb[0:kp, LB:LB + 512]),
                             start=False, stop=True, perf_mode=DRS)
            nc.tensor.matmul(gS[0:128, 0:512], pa[0:kp, :],
                             _pair(xb[0:kp, PB - 1:PB - 1 + 512]),
                             start=True, stop=False, perf_mode=DRS)
            nc.tensor.matmul(gS[0:128, 0:512], pb[0:kp, :],
                             _pair(xb[0:kp, PB:PB + 512]),
                             start=False, stop=True, perf_mode=DRS)

        # deferred work queues (software pipelining; FIFO, aged entries):
        # pool-squares 1 iter behind, adds+mul 2 behind, Sigma-m 3 behind.
        pend_psq = []     # (sq, c16, it)
        pend_add = []     # (sq, it)
        pend_sum = []     # (m, pv)
        qp_cur = None
        s_cur = None
        qp_done = []      # (qp, pv, ncols, grp)
        first_sum = [True]

        def pv_of(it):
            return BAND if it < N_IMG * N_BANDS else 64

        def emit_psq(ent):
            sq, c16, it = ent
            pv = pv_of(it)
            nc.gpsimd.tensor_tensor(
                out=sq[0:pv, EA:2048], in0=c16[0:pv, 0:NC16],
                in1=c16[0:pv, 0:NC16], op=mybir.AluOpType.mult)
            pend_add.append((sq, it))
            pend_sum.append((sq, pv))

        m2_cur = [None]

        def emit_add(ent):
            nonlocal qp_cur
            sq, it = ent
            pv = pv_of(it)
            # double-wide m tile per iteration pair: the q-mul runs once
            # per pair. Sqrt groups all start on even iterations, so a
            # pair never straddles a batch (asserted below).
            par = it % 2
            if par == 0:
                m2_cur[0] = m_pool.tile([128, 2048], F16, name="m2", tag="m")
            m2 = m2_cur[0]
            m = m2[:, 1024 * par:1024 * par + 1024]
            nc.vector.tensor_add(m[0:pv, 0:ADD_DVE], sq[0:pv, 0:ADD_DVE],
                                 sq[0:pv, 1024:1024 + ADD_DVE])
            if ADD_DVE < 1024:
                nc.gpsimd.tensor_tensor(
                    out=m[0:pv, ADD_DVE:1024], in0=sq[0:pv, ADD_DVE:1024],
                    in1=sq[0:pv, 1024 + ADD_DVE:2048], op=mybir.AluOpType.add)
            g, pos = SQRT_GRP[it], SQRT_POS[it]
            if pos == 0:
                qp_cur = qp_pool.tile([128, 4096], F16, tag="qp")
            if par == 1:
                assert pos % 2 == 1 and SQRT_GRP[it - 1] == g
                mv = m2.rearrange("p (i c) -> p i c", i=2, c=1024)
                qv = qp_cur.rearrange("p (i c) -> p i c", i=8, c=512)
                nc.vector.tensor_mul(qv[0:pv, pos - 1:pos + 1, :],
                                     mv[0:pv, :, 0:512],
                                     mv[0:pv, :, 512:1024])
            elif it == N_ITERS - 1:
                nc.vector.tensor_mul(qp_cur[0:pv, 512 * pos:512 * pos + 512],
                                     m[0:pv, 0:512], m[0:pv, 512:1024])
            if pos == SQRT_GSIZE[g] - 1:
                qp_done.append((qp_cur, pv, 0, 512 * SQRT_GSIZE[g], 2 * g))

        def emit_msum(ent):
            sq, pv = ent
            st = first_sum[0]
            first_sum[0] = False
            nc.tensor.matmul(accP[0:1, 0:512], ones[0:pv, 0:1],
                             sq[0:pv, 1536:2048],
                             start=st, stop=False, skip_group_check=True)

        def emit_sqrt(ent):
            nonlocal s_cur
            qp, pv, c0, c1, col = ent
            s_cur = s_pool.tile([128, 4096], F16, tag="s")
            nc.scalar.activation(s_cur[0:pv, 0:c1 - c0], qp[0:pv, c0:c1],
                                 mybir.ActivationFunctionType.Sqrt,
                                 accum_out=acc_s[0:pv, col:col + 1])

        emit_dma(0)
        emit_dma(1)
        emit_dma(2)
        for it in range(N_ITERS):
            pv = pv_of(it)
            xb = band_tile(it)
            gB = gb_pool.tile([128, 1536], F32, tag="gB")
            gS = gs_pool.tile([128, 512], F32, tag="gs")
            emit_mms(gB, gS, xb, it)
            if len(pend_sum) >= 3:
                emit_msum(pend_sum.pop(0))
            emit_dma(it + 3)

            # ACT evacuates gB[0:EA] as squares
            sq = sq_pool.tile([128, 2048], F16, tag="sq")
            nc.scalar.activation(sq[0:pv, 0:EA], gB[0:pv, 0:EA],
                                 mybir.ActivationFunctionType.Square,
                                 accum_out=acc_a[0:pv, it:it + 1])
            # DVE: PSUM evacuation copies first (gS frees its bank)
            c16 = c16_pool.tile([128, NC16], F16, tag="c16")
            nc.vector.tensor_copy(c16[0:pv, NSUF:NC16], gS[0:pv, 0:512])
            if NSUF:
                nc.vector.tensor_copy(c16[0:pv, 0:NSUF], gB[0:pv, EA:1536])
            if len(pend_add) >= 2:
                emit_add(pend_add.pop(0))
            if len(pend_psq) >= 2:
                emit_psq(pend_psq.pop(0))
            pend_psq.append((sq, c16, it))
            if qp_done:
                emit_sqrt(qp_done.pop(0))

        while pend_psq:
            emit_psq(pend_psq.pop(0))
        while pend_add:
            emit_add(pend_add.pop(0))
        while pend_sum:
            emit_msum(pend_sum.pop(0))
        while qp_done:
            emit_sqrt(qp_done.pop(0))
        # close the accP accumulation group
        nc.tensor.matmul(accP[0:1, 0:512], ones[0:1, 0:1], warm[0:1, 0:512],
                         start=False, stop=True, skip_group_check=True)

        if loop_ctx is not None:
            loop_ctx.__exit__(None, None, None)

        accm = acc_pool.tile([128, 512], F32, tag="accm")
        nc.scalar.activation(accm[0:1, 0:512], accP[0:1, 0:512],
                             mybir.ActivationFunctionType.Copy)
        nc.sync.dma_start(out=out[:, 0:N_ACCS], in_=acc_s[:, :])
        nc.sync.dma_start(out=out[0:1, N_ACCS:N_ACCS + 512],
                          in_=accm[0:1, 0:512])
        nc.sync.dma_start(out=out[:, N_ACCS + 512:N_ACCS + 512 + N_ITERS],
                          in_=acc_a[:, :])
    return _patch_serialization(nc)


def _prep_core_inputs(l_imgs, p_imgs, wmat):
    """l_imgs/p_imgs: [N_IMG, H, W] fp8 arrays for one core."""
    x = np.zeros((N_IMG, H + 1, XW), ml_dtypes.float8_e4m3)
    x[:, 1:, LB:LB + W] = l_imgs
    x[:, 1:, PB:PB + W] = p_imgs
    xtail = np.zeros((80, XW), ml_dtypes.float8_e4m3)
    for k in range(N_IMG):
        xtail[10 * k:10 * k + 9, LB:LB + W] = l_imgs[k, 503:512]
        xtail[10 * k:10 * k + 9, PB:PB + W] = p_imgs[k, 503:512]
    return {"x": x, "xtail": xtail, "consts": wmat}


_NC = None


def kernel(probs, labels):
    global _NC
    from concourse.bass_utils import run_bass_kernel_spmd

    if _NC is None:
        _NC = build_kernel()

    p = np.ascontiguousarray(np.asarray(probs)[:, 1:5]).astype(
        ml_dtypes.float8_e4m3)
    l = np.ascontiguousarray(np.asarray(labels)[:, 1:5]).astype(
        ml_dtypes.float8_e4m3)
    wmat = _stationaries()

    in_maps = []
    for k in range(8):
        in_maps.append(_prep_core_inputs(
            l[2 * k:2 * k + 2].reshape(N_IMG, H, W),
            p[2 * k:2 * k + 2].reshape(N_IMG, H, W), wmat))
    res = run_bass_kernel_spmd(_NC, in_maps, list(range(8)))
    total = 0.0
    for r in res.results:
        o = r["out"].astype(np.float64)
        total += (o[0, N_ACCS:N_ACCS + 512].sum()
                  + o[:, N_ACCS + 512:].sum()
                  - 2.0 * o[:, 0:N_ACCS].sum())
    return np.float32(total / (16 * H * W))


# revision 34
# speedup vs baseline: 1.1137x; 1.0010x over previous
"""BoundaryLoss kernel for 8 Trainium2 NeuronCores (v2).

loss = sum_c mean_{b,h,w}((|sobel(labels_c)| - |sobel(probs_c)|)^2)
     = (Sigma_m - 2*Sigma_sqrt(q)) / (B*H*W),   m = gx^2+gy^2, q = m_l*m_p

Data-parallel: core k processes batches [2k, 2k+1] x classes 1..4
(8 image pairs of 512x512). Partial sums are combined on host.

v2 changes vs v1:
- Inputs cast to fp8e4 on host (exact-weight Sobel taps; ~0.5% loss bias,
  well inside the 2e-2 gate). Conv matmuls use fp8 DoubleRowSwInterleave
  perf mode: each matmul contracts TWO (weights, moving) pairs at 2 rows/
  cycle, so the 4 gradient fields cost 6 half-rate matmuls instead of 9
  full-rate ones (PE time ~3x lower; HBM traffic halved).
  Moving pairs must use i-stride 2 (stride-1 pairs hard-fault the PE):
    gx   = DR(bv | -bv  ; x[c-1], x[c+1])
    gy   = DR(sdf | sdf ; x[c-1], x[c+1]) + DR(2sdf | 0 ; x[c], x[c+2])
  Weights are host-interleaved: phys[:, 2j] = W0[:, 127-j],
  phys[:, 2j+1] = W1[:, 127-j] (the documented SwInterleave layout).
- PSUM: gB = [gx_l | gx_p | gy_l] (3 banks, 2 bufs) + gS = [gy_p]
  (1 bank, single-buffered) + accP (1 bank) = 8 banks.
- Evacuation split tuned from the engine cost model: ACT Squares
  gB[0:EA] in one op; DVE copies the rest to fp16; Pool squares the
  copies (GPSIMD cannot read PSUM). m-add split DVE/Pool; q-mul on DVE.
- Sigma_m via PE ones-matmuls accumulating into accP across all
  iterations (frees ACT/DVE accum reads); Sigma_sqrt(q) via the batched
  ACT Sqrt's accum_out.
"""

import sys

import numpy as np

if "/opt/trn_rl_repo" not in sys.path:
    sys.path.insert(0, "/opt/trn_rl_repo")

from contextlib import ExitStack

import ml_dtypes

import concourse.bass as bass
import concourse.mybir as mybir
import concourse.tile as tile

H = W = 512
N_IMG = 8          # image pairs per core
BAND = 126         # output rows per full band
N_BANDS = 4        # full bands; bottom 8 rows in 1 packed tail iter
N_ITERS = N_IMG * N_BANDS + 1
XW = 1032          # [0 | L:1..512 | 0,0 | P:515..1026 | 0 pads]
LB = 1             # labels block base col
PB = 515           # probs block base col
EA = 1536          # gB cols evacuated by the ACT Square op
NSUF = 1536 - EA   # gB cols copied by DVE (gy_l tail)
NC16 = NSUF + 512  # fp16 copy width (gB suffix + gS)
ADD_DVE = 928      # m-add cols on DVE (rest on Pool)
# sqrt batches: big groups amortize overhead, small ones cut the drain
SQRT_GSIZE = [8, 8, 8, 6, 2, 1]
assert sum(SQRT_GSIZE) == N_ITERS
SQRT_GRP = [g for g, n in enumerate(SQRT_GSIZE) for _ in range(n)]
SQRT_POS = [i for n in SQRT_GSIZE for i in range(n)]
N_SQRT_G = len(SQRT_GSIZE)
N_ACCS = 2 * N_SQRT_G

F32 = mybir.dt.float32
F16 = mybir.dt.float16
F8 = mybir.dt.float8e4
DRS = mybir.MatmulPerfMode.DoubleRowSwInterleave


def _band_matrices():
    bv = np.zeros((128, 128), np.float32)   # vertical smooth [1,2,1]
    sdf = np.zeros((128, 128), np.float32)  # vertical diff [1,0,-1]
    for c in range(126):
        bv[c, c] = 1.0
        bv[c + 1, c] = 2.0
        bv[c + 2, c] = 1.0
        sdf[c, c] = 1.0
        sdf[c + 2, c] = -1.0
    # Packed tail: image k's rows 503..511 at partitions 10k..10k+8
    # (10k+9 zero halo), outputs 504..511 at partitions 8k..8k+7.
    bvm = np.zeros((128, 128), np.float32)
    sdfm = np.zeros((128, 128), np.float32)
    for k in range(8):
        for i in range(8):
            bvm[10 * k + i, 8 * k + i] = 1.0
            bvm[10 * k + i + 1, 8 * k + i] = 2.0
            bvm[10 * k + i + 2, 8 * k + i] = 1.0
            sdfm[10 * k + i, 8 * k + i] = 1.0
            sdfm[10 * k + i + 2, 8 * k + i] = -1.0
    return bv, sdf, bvm, sdfm


def _ileave(w0, w1):
    """SwInterleave weight layout: pairs interleaved, columns reversed."""
    ph = np.zeros((128, 256), np.float32)
    ph[:, 0::2] = w0[:, ::-1]
    ph[:, 1::2] = w1[:, ::-1]
    return ph


def _stationaries():
    bv, sdf, bvm, sdfm = _band_matrices()
    zz = np.zeros((128, 128), np.float32)
    blocks = [
        _ileave(bv, -bv),          # PX
        _ileave(sdf, sdf),         # PA
        _ileave(2.0 * sdf, zz),    # PB
        _ileave(bvm, -bvm),        # PXm
        _ileave(sdfm, sdfm),       # PAm
        _ileave(2.0 * sdfm, zz),   # PBm
    ]
    return np.concatenate(blocks, axis=1).astype(ml_dtypes.float8_e4m3)


def _split_waits_json(bir: bytes, maxw: int = 1) -> bytes:
    """Walrus in this container rejects instructions with >1 semaphore wait.
    Split extra waits onto NoOp carriers inserted just before the
    instruction on the same engine."""
    import orjson

    d = orjson.loads(bir)
    ctr = 0
    for fn in d["functions"]:
        for b in fn["blocks"]:
            new = []
            for ins in b["instructions"]:
                si = ins.get("sync_info")
                if si:
                    waits = si.get("on_wait") or []
                    if len(waits) > maxw:
                        keep = waits[-maxw:] if maxw else []
                        for w in waits[: len(waits) - maxw]:
                            ctr += 1
                            new.append({
                                "debug": ins.get("debug", 0),
                                "engine": ins["engine"],
                                "ins": [],
                                "outs": [],
                                "name": f"{ins['name']}-wsplit{ctr}",
                                "opcode": "NoOp",
                                "sync_info": {"on_wait": [w], "on_update": []},
                            })
                        si["on_wait"] = keep
                new.append(ins)
            b["instructions"] = new
    return orjson.dumps(d)


def _patch_serialization(nc):
    fixed = _split_waits_json(nc.to_json_bytes())
    nc.to_json_bytes = lambda: fixed
    return nc


def _pair(ap2d, istride=2):
    """[p, N] slice -> [p, 2, N] moving-pair AP (i-stride 2 only: stride-1
    pairs fault the PE in DoubleRow mode)."""
    c = ap2d.unsqueeze(1).copy()
    c.ap[1] = (istride, 2)
    return c


def build_kernel(loop: int = 1):
    nc = bass.Bass()
    x = nc.dram_tensor("x", [N_IMG, H + 1, XW], F8, kind="ExternalInput")
    xtail = nc.dram_tensor("xtail", [80, XW], F8, kind="ExternalInput")
    consts = nc.dram_tensor("consts", [128, 1536], F8, kind="ExternalInput")
    out = nc.dram_tensor("out", [128, 512 + N_ACCS], F32,
                         kind="ExternalOutput")

    with ExitStack() as ctx:
        tc = ctx.enter_context(tile.TileContext(nc))
        cpool = ctx.enter_context(tc.tile_pool(name="consts", bufs=1))
        xpool = ctx.enter_context(tc.tile_pool(name="x", bufs=1))
        gb_pool = ctx.enter_context(
            tc.tile_pool(name="gb", bufs=2, space="PSUM"))
        gs_pool = ctx.enter_context(
            tc.tile_pool(name="gs", bufs=1, space="PSUM"))
        ac_pool = ctx.enter_context(
            tc.tile_pool(name="ac", bufs=1, space="PSUM"))
        sq_pool = ctx.enter_context(tc.tile_pool(name="sq", bufs=5))
        c16_pool = ctx.enter_context(tc.tile_pool(name="c16", bufs=4))
        m_pool = ctx.enter_context(tc.tile_pool(name="m", bufs=4))
        qp_pool = ctx.enter_context(tc.tile_pool(name="qp", bufs=3))
        s_pool = ctx.enter_context(tc.tile_pool(name="s", bufs=3))
        acc_pool = ctx.enter_context(tc.tile_pool(name="acc", bufs=1))

        # PE p-state warmup while the first DMAs land.
        warm = cpool.tile([128, 512], F16, tag="warm")
        nc.gpsimd.memset(warm[:, :], 0.0)

        wmat = cpool.tile([128, 1536], F8, tag="wmat")
        nc.scalar.dma_start(out=wmat[:, :], in_=consts[:, :])
        (PX, PA, PBw, PXm, PAm, PBm) = (
            wmat[:, 256 * i:256 * i + 256] for i in range(6))

        ones = cpool.tile([128, 1], F16, tag="ones")
        nc.vector.memset(ones[:, :], 1.0)
        acc_s = acc_pool.tile([128, N_ACCS], F32, tag="acc_s")
        nc.vector.memset(acc_s[:, :], 0.0)

        N_XT = 6
        xt = [xpool.tile([128, XW], F8, name=f"x{j}", tag=f"x{j}")
              for j in range(N_XT)]
        xm = xpool.tile([128, XW], F8, tag="xm")

        gwarm = gs_pool.tile([128, 512], F32, tag="gs")
        for _ in range(7):
            nc.tensor.matmul(gwarm[0:126, 0:512], warm[0:128, 0:126],
                             warm[0:128, 0:512], start=True, stop=True)

        accP = ac_pool.tile([128, 512], F32, tag="accP")

        loop_ctx = tc.For_i(0, loop, 1) if loop > 1 else None
        if loop_ctx is not None:
            loop_ctx.__enter__()

        def band_tile(it):
            return xt[it % N_XT] if it < N_IMG * N_BANDS else xm

        def emit_dma(it):
            if it >= N_ITERS:
                return
            if it < N_IMG * N_BANDS:
                img, t = divmod(it, N_BANDS)
                nc.sync.dma_start(
                    out=xt[it % N_XT][0:128, :],
                    in_=x[img, BAND * t:BAND * t + 128, :])
            else:
                nc.sync.dma_start(out=xm[0:80, :], in_=xtail[:, :])

        def emit_mms(gB, gS, xb, it):
            if it < N_IMG * N_BANDS:
                px, pa, pb, kp = PX, PA, PBw, 128
            else:
                px, pa, pb, kp = PXm, PAm, PBm, 80
            for half, c0 in ((0, LB), (1, PB)):
                nc.tensor.matmul(gB[0:128, 512 * half:512 * half + 512],
                                 px[0:kp, :],
                                 _pair(xb[0:kp, c0 - 1:c0 - 1 + 512]),
                                 start=True, stop=True, perf_mode=DRS)
            nc.tensor.matmul(gB[0:128, 1024:1536], pa[0:kp, :],
                             _pair(xb[0:kp, LB - 1:LB - 1 + 512]),
                             start=True, stop=False, perf_mode=DRS)
            nc.tensor.matmul(gB[0:128, 1024:1536], pb[0:kp, :],
                             _pair(xb[0:kp, LB:LB + 512]),
                             start=False, stop=True, perf_mode=DRS)
            nc.tensor.matmul(gS[0:128, 0:512], pa[0:kp, :],
                             _pair(xb[0:kp, PB - 1:PB - 1 + 512]),
                             start=True, stop=False, perf_mode=DRS)
            nc.tensor.matmul(gS[0:128, 0:512], pb[0:kp, :],
                             _pair(xb[0:kp, PB:PB + 512]),
                             start=False, stop=True, perf_mode=DRS)

        # deferred work queues (software pipelining; FIFO, aged entries):
        # pool-squares 1 iter behind, adds+mul 2 behind, Sigma-m 3 behind.
        pend_psq = []     # (sq, c16, it)
        pend_add = []     # (sq, it)
        pend_sum = []     # (m, pv)
        qp_cur = None
        s_cur = None
        qp_done = []      # (qp, pv, ncols, grp)
        first_sum = [True]

        def pv_of(it):
            return BAND if it < N_IMG * N_BANDS else 64

        def emit_psq(ent):
            sq, c16, it = ent
            pv = pv_of(it)
            nc.gpsimd.tensor_tensor(
                out=sq[0:pv, EA:2048], in0=c16[0:pv, 0:NC16],
                in1=c16[0:pv, 0:NC16], op=mybir.AluOpType.mult)
            pend_add.append((sq, it))

        m2_cur = [None]

        def emit_add(ent):
            nonlocal qp_cur
            sq, it = ent
            pv = pv_of(it)
            # double-wide m tile per iteration pair: the q-mul runs once
            # per pair. Sqrt groups all start on even iterations, so a
            # pair never straddles a batch (asserted below).
            par = it % 2
            if par == 0:
                m2_cur[0] = m_pool.tile([128, 2048], F16, name="m2", tag="m")
            m2 = m2_cur[0]
            m = m2[:, 1024 * par:1024 * par + 1024]
            nc.vector.tensor_add(m[0:pv, 0:ADD_DVE], sq[0:pv, 0:ADD_DVE],
                                 sq[0:pv, 1024:1024 + ADD_DVE])
            if ADD_DVE < 1024:
                nc.gpsimd.tensor_tensor(
                    out=m[0:pv, ADD_DVE:1024], in0=sq[0:pv, ADD_DVE:1024],
                    in1=sq[0:pv, 1024 + ADD_DVE:2048], op=mybir.AluOpType.add)
            g, pos = SQRT_GRP[it], SQRT_POS[it]
            if pos == 0:
                qp_cur = qp_pool.tile([128, 4096], F16, tag="qp")
            if par == 1:
                assert pos % 2 == 1 and SQRT_GRP[it - 1] == g
                mv = m2.rearrange("p (i c) -> p i c", i=2, c=1024)
                qv = qp_cur.rearrange("p (i c) -> p i c", i=8, c=512)
                nc.vector.tensor_mul(qv[0:pv, pos - 1:pos + 1, :],
                                     mv[0:pv, :, 0:512],
                                     mv[0:pv, :, 512:1024])
            elif it == N_ITERS - 1:
                nc.vector.tensor_mul(qp_cur[0:pv, 512 * pos:512 * pos + 512],
                                     m[0:pv, 0:512], m[0:pv, 512:1024])
            if pos == SQRT_GSIZE[g] - 1:
                qp_done.append((qp_cur, pv, 0, 512 * SQRT_GSIZE[g], 2 * g))
            pend_sum.append((m, pv))

        def emit_msum(ent):
            m, pv = ent
            for hh in (0, 1):
                st = first_sum[0]
                first_sum[0] = False
                nc.tensor.matmul(accP[0:1, 0:512], ones[0:pv, 0:1],
                                 m[0:pv, 512 * hh:512 * hh + 512],
                                 start=st, stop=False, skip_group_check=True)

        def emit_sqrt(ent):
            nonlocal s_cur
            qp, pv, c0, c1, col = ent
            s_cur = s_pool.tile([128, 4096], F16, tag="s")
            nc.scalar.activation(s_cur[0:pv, 0:c1 - c0], qp[0:pv, c0:c1],
                                 mybir.ActivationFunctionType.Sqrt,
                                 accum_out=acc_s[0:pv, col:col + 1])

        emit_dma(0)
        emit_dma(1)
        emit_dma(2)
        for it in range(N_ITERS):
            pv = pv_of(it)
            xb = band_tile(it)
            gB = gb_pool.tile([128, 1536], F32, tag="gB")
            gS = gs_pool.tile([128, 512], F32, tag="gs")
            emit_mms(gB, gS, xb, it)
            if len(pend_sum) >= 3:
                emit_msum(pend_sum.pop(0))
            emit_dma(it + 3)

            # ACT evacuates gB[0:EA] as squares
            sq = sq_pool.tile([128, 2048], F16, tag="sq")
            nc.scalar.activation(sq[0:pv, 0:EA], gB[0:pv, 0:EA],
                                 mybir.ActivationFunctionType.Square)
            # DVE: PSUM evacuation copies first (gS frees its bank)
            c16 = c16_pool.tile([128, NC16], F16, tag="c16")
            nc.vector.tensor_copy(c16[0:pv, NSUF:NC16], gS[0:pv, 0:512])
            if NSUF:
                nc.vector.tensor_copy(c16[0:pv, 0:NSUF], gB[0:pv, EA:1536])
            if len(pend_add) >= 2:
                emit_add(pend_add.pop(0))
            if len(pend_psq) >= 2:
                emit_psq(pend_psq.pop(0))
            pend_psq.append((sq, c16, it))
            if qp_done:
                emit_sqrt(qp_done.pop(0))

        while pend_psq:
            emit_psq(pend_psq.pop(0))
        while pend_add:
            emit_add(pend_add.pop(0))
        while pend_sum:
            emit_msum(pend_sum.pop(0))
        while qp_done:
            emit_sqrt(qp_done.pop(0))
        # close the accP accumulation group
        nc.tensor.matmul(accP[0:1, 0:512], ones[0:1, 0:1], warm[0:1, 0:512],
                         start=False, stop=True, skip_group_check=True)

        if loop_ctx is not None:
            loop_ctx.__exit__(None, None, None)

        accm = acc_pool.tile([128, 512], F32, tag="accm")
        nc.scalar.activation(accm[0:1, 0:512], accP[0:1, 0:512],
                             mybir.ActivationFunctionType.Copy)
        nc.sync.dma_start(out=out[:, 0:N_ACCS], in_=acc_s[:, :])
        nc.sync.dma_start(out=out[0:1, N_ACCS:N_ACCS + 512],
                          in_=accm[0:1, 0:512])
    return _patch_serialization(nc)


def _prep_core_inputs(l_imgs, p_imgs, wmat):
    """l_imgs/p_imgs: [N_IMG, H, W] fp8 arrays for one core."""
    x = np.zeros((N_IMG, H + 1, XW), ml_dtypes.float8_e4m3)
    x[:, 1:, LB:LB + W] = l_imgs
    x[:, 1:, PB:PB + W] = p_imgs
    xtail = np.zeros((80, XW), ml_dtypes.float8_e4m3)
    for k in range(N_IMG):
        xtail[10 * k:10 * k + 9, LB:LB + W] = l_imgs[k, 503:512]
        xtail[10 * k:10 * k + 9, PB:PB + W] = p_imgs[k, 503:512]
    return {"x": x, "xtail": xtail, "consts": wmat}


_NC = None


def kernel(probs, labels):
    global _NC
    from concourse.bass_utils import run_bass_kernel_spmd

    if _NC is None:
        _NC = build_kernel()

    p = np.ascontiguousarray(np.asarray(probs)[:, 1:5]).astype(
        ml_dtypes.float8_e4m3)
    l = np.ascontiguousarray(np.asarray(labels)[:, 1:5]).astype(
        ml_dtypes.float8_e4m3)
    wmat = _stationaries()

    in_maps = []
    for k in range(8):
        in_maps.append(_prep_core_inputs(
            l[2 * k:2 * k + 2].reshape(N_IMG, H, W),
            p[2 * k:2 * k + 2].reshape(N_IMG, H, W), wmat))
    res = run_bass_kernel_spmd(_NC, in_maps, list(range(8)))
    total = 0.0
    for r in res.results:
        o = r["out"].astype(np.float64)
        total += (o[0, N_ACCS:].sum() - 2.0 * o[:, 0:N_ACCS].sum())
    return np.float32(total / (16 * H * W))
